# revision 11
# baseline (speedup 1.0000x reference)
"""Trainium2 Bass kernel for CausalSelfAttention (B=2, T=2048, D=1024, H=16).

Sharding (8 cores): Megatron-style tensor parallel. Core c owns heads
{2c, 2c+1}: column-parallel c_attn (384 of 3072 output features),
full attention for its 2 heads x 2 batches, row-parallel c_proj
(128 of 1024 contraction rows). Host sums the 8 partial outputs and
adds b_proj (+ exact b_v correction).

v2 structure notes:
  - Emission interleaves the qkv projection (per 1024-token group) with
    the attention spans that depend on it, so engine FIFOs pipeline
    across phases instead of serializing them.
  - xT arrives host-side pre-masked (masked token columns zeroed), so
    masked keys have v == 0 on device; the denominator-exclusion mask
    column (mask + 1e-18) is bulk-filled into v_nat once.  Masked
    queries produce huge-but-finite garbage rows that are zeroed by a
    per-partition query-mask multiply fused into the c_proj PSUM
    eviction (replaces the old mrowinv/den-add path).
  - QK matmuls for the two heads go to PE row groups 0-63 / 64-127
    (via qT/kT partition layout) and are issued dj-major so pairs run
    concurrently in the array.
  - PSUM: one 3-slot [128,1024] ring shared by QK S-tiles and the qkv
    pq tiles (6 banks) + one 2-slot 1-bank ring shared by PV
    accumulators, proj po tiles and v-transpose tiles (2 banks).
"""

import functools

import numpy as np
import ml_dtypes

import concourse.bass as bass
import concourse.mybir as mybir
import concourse.tile as tile
from concourse import bacc
from concourse.bass_utils import run_bass_kernel_spmd
from concourse.masks import make_upper_triangular, make_identity

BF16 = mybir.dt.bfloat16
F32 = mybir.dt.float32
AF = mybir.ActivationFunctionType
OP = mybir.AluOpType

B, T, D, NH = 2, 2048, 1024, 16
DH = 64                  # head dim
HPC = 2                  # heads per core
NCORES = 8
TT = B * T               # 4096 total tokens
P = 128
KC = D // P              # 8 contraction tiles for qkv
SPAN = 512               # q-span processed per softmax pass
NSP = T // SPAN          # 4 spans per batch
NKT = T // P             # 16 k-tiles per batch
NTLE = TT // P           # 32 token tiles total
QSCALE = 1.0 / np.sqrt(DH)
ESHIFT = -10.0           # constant exp shift; cancels in softmax ratio
VW = 2 * DH + 2          # v_nat row width: [h0 v | m | h1 v | m]


def build(debug_outs=False):
    nc = bacc.Bacc(None)

    xT = nc.dram_tensor("xT", [D, TT], BF16, kind="ExternalInput")
    wqkv = nc.dram_tensor("wqkv", [KC, P, 3 * P], BF16, kind="ExternalInput")
    bqkv = nc.dram_tensor("bqkv", [P, 3], F32, kind="ExternalInput")
    wproj = nc.dram_tensor("wproj", [P, D], BF16, kind="ExternalInput")
    mcolden = nc.dram_tensor("mcolden", [P, NTLE, 1], F32, kind="ExternalInput")
    mcol01 = nc.dram_tensor("mcol01", [P, NTLE, 1], F32, kind="ExternalInput")
    out = nc.dram_tensor("out", [TT, D], BF16, kind="ExternalOutput")

    with tile.TileContext(nc) as tc:
        with (
            tc.tile_pool(name="singles", bufs=1) as singles,
            tc.tile_pool(name="stage", bufs=2) as stage,
            tc.tile_pool(name="pt", bufs=2) as ptp,
            tc.tile_pool(name="rows", bufs=2) as rows,
            tc.tile_pool(name="outs", bufs=3) as outs,
            tc.tile_pool(name="st", bufs=3, space="PSUM") as ps_st,
            tc.tile_pool(name="acc", bufs=2, space="PSUM") as ps_acc,
        ):
            # ---- constants / weights ----
            wqkv_sb = singles.tile([P, KC, 3 * P], BF16)
            nc.sync.dma_start(out=wqkv_sb, in_=wqkv.rearrange("k p m -> p k m"))
            bqkv_sb = singles.tile([P, 3], F32)
            nc.sync.dma_start(out=bqkv_sb, in_=bqkv[:, :])
            wproj_sb = singles.tile([P, D], BF16)
            nc.sync.dma_start(out=wproj_sb, in_=wproj[:, :])
            mden_sb = singles.tile([P, NTLE, 1], F32)
            nc.sync.dma_start(out=mden_sb, in_=mcolden[:, :, :])
            m01_sb = singles.tile([P, NTLE, 1], F32)
            nc.sync.dma_start(out=m01_sb, in_=mcol01[:, :, :])

            eshift_sb = singles.tile([P, 1], F32)
            nc.vector.memset(eshift_sb, ESHIFT)
            ut_sb = singles.tile([P, P], BF16)  # keep q >= k
            make_upper_triangular(nc, ut_sb, val=1.0, diag=True)
            ident = singles.tile([P, P], BF16)
            make_identity(nc, ident)

            xT_sb = singles.tile([P, KC, TT], BF16)
            qT_sb = singles.tile([P, TT], BF16)   # rows: h0 d0..63 | h1 d0..63
            kT_sb = singles.tile([P, TT], BF16)
            yT_sb = singles.tile([P, TT], BF16)
            v_nat = singles.tile([P, B * NKT, VW], BF16)

            # denominator-mask columns for all 32 k-tiles in two bulk ops
            nc.vector.tensor_copy(out=v_nat[:, :, DH:DH + 1], in_=mden_sb)
            nc.vector.tensor_copy(out=v_nat[:, :, VW - 1:VW], in_=mden_sb)

            def qkv_group(n2):
                tsl = slice(n2 * 1024, (n2 + 1) * 1024)
                for k in range(KC):
                    nc.sync.dma_start(out=xT_sb[:, k, tsl],
                                      in_=xT[k * P:(k + 1) * P, tsl])
                for m in range(3):
                    pq = ps_st.tile([P, 1024], F32, tag="st")
                    for k in range(KC):
                        for h2 in range(2):
                            nc.tensor.matmul(
                                pq[:, h2 * 512:(h2 + 1) * 512],
                                wqkv_sb[:, k, m * P:(m + 1) * P],
                                xT_sb[:, k, n2 * 1024 + h2 * 512:
                                      n2 * 1024 + (h2 + 1) * 512],
                                start=(k == 0), stop=(k == KC - 1),
                            )
                    if m == 0:
                        nc.scalar.activation(
                            qT_sb[:, tsl], pq[:], AF.Identity,
                            bias=bqkv_sb[:, 0:1], scale=QSCALE)
                    elif m == 1:
                        nc.scalar.activation(
                            kT_sb[:, tsl], pq[:], AF.Identity,
                            bias=bqkv_sb[:, 1:2], scale=1.0)
                    else:
                        vst = stage.tile([P, 1024], BF16, tag="vst")
                        nc.scalar.activation(
                            vst[:], pq[:], AF.Identity,
                            bias=bqkv_sb[:, 2:3], scale=1.0)
                        for jj in range(1024 // P):
                            j32 = n2 * 8 + jj
                            vtp = ps_acc.tile([P, P], BF16, tag="acc")
                            nc.tensor.transpose(
                                vtp[:], vst[:, jj * P:(jj + 1) * P], ident[:])
                            nc.vector.tensor_copy(
                                out=v_nat[:, j32, 0:DH], in_=vtp[:, 0:DH])
                            nc.vector.tensor_copy(
                                out=v_nat[:, j32, DH + 1:2 * DH + 1],
                                in_=vtp[:, DH:2 * DH])

            def attn_span(b, s):
                qg = b * T + s * SPAN
                njs = 4 * (s + 1)
                pvs = [ps_acc.tile([DH + 1, SPAN], F32, tag="acc",
                                   name=f"pv{b}_{s}_{_h}") for _h in range(HPC)]
                for jj in range(0, njs, 2):
                    sts = [ps_st.tile([P, 1024], F32, tag="st",
                                      name=f"st{_h}") for _h in range(HPC)]
                    pts = [ptp.tile([P, 1024], BF16, tag=f"pt{_h}",
                                    name=f"pt{_h}") for _h in range(HPC)]
                    offs = [max(0, jj + dj - 4 * s) * P for dj in range(2)]
                    # QK: dj-major, h-minor so the two heads' matmuls land
                    # in disjoint PE row groups back-to-back (concurrent)
                    for dj in range(2):
                        j = jj + dj
                        off = offs[dj]
                        kb = b * T + j * P
                        for h in range(HPC):
                            hb = h * DH
                            nc.tensor.matmul(
                                sts[h][:, dj * 512 + off:(dj + 1) * 512],
                                kT_sb[hb:hb + DH, kb:kb + P],
                                qT_sb[hb:hb + DH, qg + off:qg + SPAN],
                                start=True, stop=True,
                            )
                    for h in range(HPC):
                        nc.scalar.activation(
                            pts[h][:, offs[0]:1024], sts[h][:, offs[0]:1024],
                            AF.Exp, bias=eshift_sb[:])
                    for dj in range(2):
                        j = jj + dj
                        if j >= 4 * s:  # diagonal block: keep q >= k
                            off = offs[dj]
                            dsl = slice(dj * 512 + off, dj * 512 + off + P)
                            for h in range(HPC):
                                nc.vector.tensor_tensor(
                                    pts[h][:, dsl], pts[h][:, dsl], ut_sb[:],
                                    OP.mult)
                    for dj in range(2):
                        j = jj + dj
                        off = offs[dj]
                        for h in range(HPC):
                            vc0 = h * (DH + 1)
                            nc.tensor.matmul(
                                pvs[h][:, off:SPAN],
                                v_nat[:, b * NKT + j, vc0:vc0 + DH + 1],
                                pts[h][:, dj * 512 + off:(dj + 1) * 512],
                                start=(j == 0), stop=(j == njs - 1),
                            )
                # tail: normalize into yT (unmasked-query garbage rows are
                # huge-but-finite; zeroed at proj eviction by mcol01)
                for h in range(HPC):
                    # lhsT cols [v|m] for both heads: v rows 0..63, denom 64
                    # (recip_approx_fast misbehaves on PSUM input at partition
                    # base 64 — stage the denominator row into SBUF first)
                    den = rows.tile([1, SPAN], F32, tag="den")
                    nc.vector.tensor_copy(out=den, in_=pvs[h][DH:DH + 1, :])
                    rq = rows.tile([1, SPAN], F32, tag="rq")
                    nc.vector.reciprocal_approx_fast(out=rq, in_=den)
                    bc_sb = rows.tile([DH, SPAN], F32, tag="bcs")
                    nc.gpsimd.partition_broadcast(bc_sb[:], rq[:])
                    hb = h * DH
                    nc.vector.tensor_tensor(
                        yT_sb[hb:hb + DH, qg:qg + SPAN],
                        pvs[h][0:DH, :], bc_sb[:], OP.mult)
                for tt in range(qg // P, (qg + SPAN) // P):
                    ob = outs.tile([P, D], BF16, tag="ob")
                    for half in range(2):
                        po = ps_acc.tile([P, 512], F32, tag="acc", name="po")
                        nc.tensor.matmul(
                            po[:],
                            yT_sb[:, tt * P:(tt + 1) * P],
                            wproj_sb[:, half * 512:(half + 1) * 512],
                            start=True, stop=True,
                        )
                        nc.vector.tensor_scalar_mul(
                            ob[:, half * 512:(half + 1) * 512], po[:],
                            m01_sb[:, tt, :])
                    nc.sync.dma_start(out=out[tt * P:(tt + 1) * P, :], in_=ob)

            # interleaved emission: each qkv token-group followed by the
            # attention spans it unblocks
            qkv_group(0)
            attn_span(0, 0)
            attn_span(0, 1)
            qkv_group(1)
            attn_span(0, 2)
            attn_span(0, 3)
            qkv_group(2)
            attn_span(1, 0)
            attn_span(1, 1)
            qkv_group(3)
            attn_span(1, 2)
            attn_span(1, 3)

            if debug_outs:
                d_qT = nc.dram_tensor("d_qT", [P, TT], BF16,
                                      kind="ExternalOutput")
                d_kT = nc.dram_tensor("d_kT", [P, TT], BF16,
                                      kind="ExternalOutput")
                d_yT = nc.dram_tensor("d_yT", [P, TT], BF16,
                                      kind="ExternalOutput")
                d_vn = nc.dram_tensor("d_vn", [P, B * NKT * VW], BF16,
                                      kind="ExternalOutput")
                nc.sync.dma_start(out=d_qT[:, :], in_=qT_sb)
                nc.sync.dma_start(out=d_kT[:, :], in_=kT_sb)
                nc.sync.dma_start(out=d_yT[:, :], in_=yT_sb)
                nc.sync.dma_start(
                    out=d_vn.rearrange("p (j w) -> p j w", w=VW), in_=v_nat)

    nc.finalize()
    return nc


@functools.lru_cache(maxsize=1)
def _built():
    return build()


def _prep_core(c, W_attn, b_attn, W_proj):
    bf = ml_dtypes.bfloat16
    q0 = c * HPC * DH
    qs = slice(q0, q0 + P)
    ks = slice(D + q0, D + q0 + P)
    vs = slice(2 * D + q0, 2 * D + q0 + P)
    wsl = np.concatenate(
        [W_attn[:, qs], W_attn[:, ks], W_attn[:, vs]], axis=1)  # [1024, 384]
    bq = b_attn[qs] * QSCALE
    return {
        "wqkv": np.ascontiguousarray(wsl.reshape(KC, P, 3 * P)).astype(bf),
        "bqkv": np.ascontiguousarray(
            np.stack([bq, b_attn[ks], b_attn[vs]], axis=1)).astype(np.float32),
        "wproj": np.ascontiguousarray(W_proj[qs, :]).astype(bf),
    }


def build_in_maps(x, attention_mask, W_attn, b_attn, W_proj):
    bf = ml_dtypes.bfloat16
    x = np.asarray(x, dtype=np.float32)
    attention_mask = np.asarray(attention_mask)
    W_attn = np.asarray(W_attn, dtype=np.float32)
    b_attn = np.asarray(b_attn, dtype=np.float32)
    W_proj = np.asarray(W_proj, dtype=np.float32)

    maskf = attention_mask.astype(np.float32)
    xm = x * maskf[:, :, None]
    xT = np.ascontiguousarray(xm.reshape(TT, D).T).astype(bf)
    # [p, b*NKT+o, 1] with token t = b*T + o*128 + p
    mtile = np.ascontiguousarray(
        maskf.reshape(B * NKT, P).T[:, :, None]).astype(np.float32)
    mden = (mtile + 1e-18).astype(np.float32)

    in_maps = []
    for c in range(NCORES):
        m = _prep_core(c, W_attn, b_attn, W_proj)
        m["xT"] = xT
        m["mcolden"] = mden
        m["mcol01"] = mtile
        in_maps.append(m)
    return in_maps


def kernel(x, attention_mask, W_attn, b_attn, W_proj, b_proj):
    b_proj = np.asarray(b_proj, dtype=np.float32)
    b_attn = np.asarray(b_attn, dtype=np.float32)
    W_proj_f = np.asarray(W_proj, dtype=np.float32)
    nc = _built()
    in_maps = build_in_maps(x, attention_mask, W_attn, b_attn, W_proj)
    res = run_bass_kernel_spmd(nc, in_maps, core_ids=list(range(NCORES)))
    acc = np.zeros((TT, D), dtype=np.float32)
    for c in range(NCORES):
        acc += res.results[c]["out"].astype(np.float32)
    # v-bias correction: device computed y from bias-free v (xT pre-masked);
    # y_true = y0 + b_v for valid queries, so add qmask x (b_v @ W_proj)
    bv = b_attn[2 * D:3 * D]
    corr = bv @ W_proj_f  # [1024]
    qmask = np.asarray(attention_mask).astype(np.float32).reshape(TT)
    acc += qmask[:, None] * corr[None, :]
    acc += b_proj[None, :]
    return acc.reshape(B, T, D)
